# revision 37
# baseline (speedup 1.0000x reference)
"""TRN2 Bass kernel for nn_DiffQuantumSimulator (QAOA MaxCut, 18 qubits, p=4).

Data-parallel over batch (8 graphs -> 8 NeuronCores). Per core the 2^18
statevector lives in SBUF as fp16 [128 partitions x (2 planes x 2048)]
(re/im planes).

Each QAOA layer: diagonal phase exp(-i*hp) (elementwise) + mixer
RX(beta)^(x)18 done in 3 fp16 matmul stages (7+7+4 qubits).

Stages A and B use the state as the stationary operand, fusing a
partition<->free transpose into the matmul; complex arithmetic is two
accumulating matmuls per window against [C|D] and [-D|C] gate blocks.
Stage C applies kron(RX^4, I8) with the gate stationary. All PSUM
evictions write CONTIGUOUS SBUF ranges (strided SBUF writes are ~4x
slower); the layout permutation is carried by stage B's strided
stationary reads (8-element runs) instead. The per-layer bit
permutation tau (blocks 7-10 <-> 14-17) is an involution, so layouts
alternate tau/identity and the energy layout is the identity.

The diagonal rotation exp(-i*hp) is applied WITHOUT combine passes: one
PSUM->fp16 eviction + two fp16 product passes produce the four planes
(re*c, im*s, re*s, im*c), and the next layer's stage A consumes them
directly as four accumulating matmuls per window (matmul linearity; the
subtraction rides in a sign-flipped gate block [D|-C]). This removes the
rotation's exposed latency from the critical path entirely. Energy =
sum(|amp|^2 * hp/64) via ACT Square + DVE STT accumulate; host scales
by 64/DIM. Input DMAs are ordered by first use across the three serial
engine queues (~110GB/s each) so the first matmul starts ~1us after the
fixed ~9.5us framework startup.
"""

import numpy as np

import concourse.bass as bass
import concourse.mybir as mybir
import concourse.tile as tile
from concourse import bacc
from concourse.bass_utils import run_bass_kernel_spmd

N = 18
DIM = 1 << N
P = 128
F = 2048
LAYERS = 4
BATCH = 8
NCORES = 8
HP_SCALE = 64.0
GW = 1152  # per layer: [C7|D7] [-D7|C7] [D7|-C7] (3x256) C4 -D4 D4 (3x128)

FP32 = mybir.dt.float32
FP16 = mybir.dt.float16
ALU = mybir.AluOpType

# ----------------------------------------------------------------------------
# Host-side math
# ----------------------------------------------------------------------------


def _compute_hp(adj):
    W = (np.triu(adj, k=1) > 0.5).astype(np.float64)
    n_edges = W.sum()
    idx = np.arange(DIM)
    shifts = (N - 1 - np.arange(N))[:, None]
    Z = 1.0 - 2.0 * ((idx[None, :] >> shifts) & 1).astype(np.float64)
    T = W @ Z
    cross = np.einsum("ud,ud->d", T, Z)
    return 0.5 * (n_edges - cross)  # [DIM], integer-valued, exact


def _tau(b):
    """Per-layer bit-position permutation: A moves 11-17 -> 0-6 and 0-6 ->
    11-17 keeping 7-10; B (stride-16 windows, out part = col/16) then maps
    0-3 -> 7-10, 4-6 -> 11-13, 7-10 -> 14-17, 11-17 -> 0-6. Composition:"""
    if b <= 6:
        return b
    if b <= 10:
        return b + 7
    return b - 4


def _perm_for_power(s):
    pos = list(range(N))
    for _ in range(s):
        pos = [_tau(p) for p in pos]
    y = np.arange(DIM, dtype=np.int64)
    x = np.zeros(DIM, dtype=np.int64)
    for b in range(N):
        x |= ((y >> pos[b]) & 1) << b
    return x


_PERMS = [_perm_for_power(s) for s in range(5)]


def _rx(beta):
    c, s = np.cos(beta), np.sin(beta)
    return np.array([[c, -1j * s], [-1j * s, c]], dtype=np.complex128)


def _kron_list(mats):
    out = np.array([[1.0]], dtype=np.complex128)
    for m in mats:
        out = np.kron(out, m)
    return out


def _host_prep(batch_betas, adj_matrices):
    in_maps = []
    for b in range(BATCH):
        hp = _compute_hp(np.asarray(adj_matrices[b], dtype=np.float64))

        # init state: e^{-i hp} in identity layout (layer-1 rotation folded in)
        init = np.empty((P, 2, F), dtype=np.float64)
        init[:, 0, :] = np.cos(hp).reshape(P, F)
        init[:, 1, :] = (-np.sin(hp)).reshape(P, F)
        init = init.reshape(P, 2 * F).astype(np.float16)

        # rotation diags: layer 0 and 2 rotations share the tau layout,
        # layer 1 uses identity. Per chunk k: [cos|sin|cos] (1536 cols) so
        # (cos|sin) and (sin|cos) are both contiguous slices.
        def _csc(h):
            hl = h.reshape(P, 4, 512)
            c, s = np.cos(hl), np.sin(hl)
            return np.concatenate([c, s, c], axis=2).reshape(P, 6144).astype(np.float16)

        cs_tau = _csc(hp[_PERMS[1]])
        cs_id = _csc(hp)

        hp_fin = (hp[_PERMS[4]].reshape(P, F) / HP_SCALE).astype(np.float16)

        gates = np.empty((P, LAYERS * GW), dtype=np.float64)
        for t in range(LAYERS):
            beta = float(np.asarray(batch_betas[b][t], dtype=np.float64))
            M7 = _kron_list([_rx(beta)] * 7)
            C7, D7 = M7.real, M7.imag
            MC = np.kron(_kron_list([_rx(beta)] * 4), np.eye(8))
            C4, D4 = MC.real, MC.imag
            g = gates[:, GW * t : GW * (t + 1)]
            g[:, 0:128] = C7
            g[:, 128:256] = D7
            g[:, 256:384] = -D7
            g[:, 384:512] = C7
            g[:, 512:640] = D7
            g[:, 640:768] = -C7
            g[:, 768:896] = C4
            g[:, 896:1024] = -D4
            g[:, 1024:1152] = D4
        gates = gates.astype(np.float16)

        in_maps.append(
            {"init": init, "gates": gates, "cs_tau": cs_tau, "cs_id": cs_id,
             "hp_fin": hp_fin}
        )
    return in_maps


# ----------------------------------------------------------------------------
# Bass program
# ----------------------------------------------------------------------------


def _build_program():
    nc = bacc.Bacc("TRN2", target_bir_lowering=False, debug=False)

    d_init = nc.dram_tensor("init", [P, 2 * F], FP16, kind="ExternalInput")
    d_gates = nc.dram_tensor("gates", [P, LAYERS * GW], FP16, kind="ExternalInput")
    d_cs_tau = nc.dram_tensor("cs_tau", [P, 6144], FP16, kind="ExternalInput")
    d_cs_id = nc.dram_tensor("cs_id", [P, 6144], FP16, kind="ExternalInput")
    d_hp = nc.dram_tensor("hp_fin", [P, F], FP16, kind="ExternalInput")
    d_out = nc.dram_tensor("out", [P, 1], FP32, kind="ExternalOutput")

    with tile.TileContext(nc) as tc:
        with (
            tc.tile_pool(name="state", bufs=1) as st_pool,
            tc.tile_pool(name="consts", bufs=1) as c_pool,
            tc.tile_pool(name="sc", bufs=3) as sc_pool,
            tc.tile_pool(name="pr", bufs=3) as pr_pool,
            tc.tile_pool(name="misc", bufs=1) as m_pool,
            tc.tile_pool(name="ps_ab", bufs=4, space="PSUM") as ps_ab,
            tc.tile_pool(name="ps_c", bufs=2, space="PSUM") as ps_c,
        ):
            st_x = st_pool.tile([P, 2 * F], FP16, tag="st_x")
            st_y = st_pool.tile([P, 2 * F], FP16, tag="st_y")
            gates = c_pool.tile([P, LAYERS * GW], FP16, tag="gates")
            cs_tau = c_pool.tile([P, 6144], FP16, tag="cs_tau")
            cs_id = c_pool.tile([P, 6144], FP16, tag="cs_id")
            hp_d = c_pool.tile([P, F], FP16, tag="hp")
            rp = c_pool.tile([P, 8192], FP16, tag="rp")
            re_c = c_pool.tile([P, 2048], FP16, tag="re_c")

            part_k = [
                m_pool.tile([P, 1], FP32, tag=f"part{k}", name=f"part{k}")
                for k in range(4)
            ]
            red0 = m_pool.tile([P, 1], FP32, tag="red0")
            red1 = m_pool.tile([P, 1], FP32, tag="red1")
            partial = m_pool.tile([P, 1], FP32, tag="partial")

            # ---- input DMAs: per-engine queues are serial (~110GB/s),
            # so order pieces by first use. The three critical first pieces
            # (init re k0, init im k0, layer-1 A-gates) go first on the three
            # queues; cs_tau is split three ways right behind.
            nc.sync.dma_start(st_x[:, 0:256], d_init.ap()[:, 0:256])
            nc.sync.dma_start(st_x[:, 256:1024], d_init.ap()[:, 256:1024])
            nc.sync.dma_start(st_x[:, 1024:2048], d_init.ap()[:, 1024:2048])
            nc.sync.dma_start(cs_tau[:, 0:2048], d_cs_tau.ap()[:, 0:2048])
            nc.sync.dma_start(cs_id[:, 0:3072], d_cs_id.ap()[:, 0:3072])
            nc.sync.dma_start(gates[:, GW : 2 * GW], d_gates.ap()[:, GW : 2 * GW])

            nc.scalar.dma_start(gates[:, 0:512], d_gates.ap()[:, 0:512])
            nc.scalar.dma_start(gates[:, 512:GW], d_gates.ap()[:, 512:GW])
            nc.scalar.dma_start(cs_tau[:, 2048:4096], d_cs_tau.ap()[:, 2048:4096])
            nc.scalar.dma_start(cs_id[:, 3072:6144], d_cs_id.ap()[:, 3072:6144])
            nc.scalar.dma_start(
                gates[:, 2 * GW : 4 * GW], d_gates.ap()[:, 2 * GW : 4 * GW]
            )
            nc.scalar.dma_start(hp_d[:], d_hp.ap())

            nc.gpsimd.dma_start(st_x[:, 2048:2304], d_init.ap()[:, 2048:2304])
            nc.gpsimd.dma_start(st_x[:, 2304:3072], d_init.ap()[:, 2304:3072])
            nc.gpsimd.dma_start(st_x[:, 3072:4096], d_init.ap()[:, 3072:4096])
            nc.gpsimd.dma_start(cs_tau[:, 4096:6144], d_cs_tau.ap()[:, 4096:6144])

            stx3 = st_x[:].rearrange("p (two c) -> p two c", two=2)
            sty3 = st_y[:].rearrange("p (two c) -> p two c", two=2)
            # phase-B stationary views: [p, v(16; stride 1), j(128; stride 16)]
            sty_b = [
                st_y[:, 2048 * pl : 2048 * (pl + 1)].rearrange(
                    "p (j v) -> p v j", v=16
                )
                for pl in (0, 1)
            ]

            def evict_ab2(src_ps, dst3, g, engine):
                """[P,512] PSUM group (2 windows) -> contiguous cols
                256g+128j+n. engine='both' splits re/im across ACT+DVE to
                minimize the barrier tail on the last group."""
                src = src_ps[:].rearrange("p (j two n) -> p two j n", j=2, two=2)
                dst = dst3[:, :, 256 * g : 256 * (g + 1)].rearrange(
                    "p two (j n) -> p two j n", j=2
                )
                if engine == "act":
                    nc.scalar.copy(dst, src)
                elif engine == "dve":
                    nc.vector.tensor_copy(dst, src)
                else:
                    nc.scalar.copy(dst[:, 0], src[:, 0])
                    nc.vector.tensor_copy(dst[:, 1], src[:, 1])

            # Pool warm-up: force the compute library load early
            nc.gpsimd.tensor_tensor(red0[:], part_k[0][:], part_k[0][:], ALU.add)

            for t in range(LAYERS):
                gb = GW * t
                g_cd = gates[:, gb : gb + 256]
                g_nc = gates[:, gb + 256 : gb + 512]
                g_ncn = gates[:, gb + 512 : gb + 768]
                g_c4 = gates[:, gb + 768 : gb + 896]
                g_nd4 = gates[:, gb + 896 : gb + 1024]
                g_d4 = gates[:, gb + 1024 : gb + 1152]

                # ---- phase A: stationary = state windows (fused transpose).
                # Layer 0 reads the init state (2 matmuls/window); later
                # layers read the uncombined rotation product planes
                # (4 accumulating matmuls/window) so no combine pass exists.
                for g in range(8):
                    ps = ps_ab.tile([P, 512], FP32, tag="ps_ab")
                    for j in range(2):
                        w = 2 * g + j
                        out = ps[:, 256 * j : 256 * (j + 1)]
                        if t == 0:
                            win = slice(128 * w, 128 * (w + 1))
                            nc.tensor.matmul(
                                out, stx3[:, 0, win], g_cd, start=True, stop=False
                            )
                            nc.tensor.matmul(
                                out, stx3[:, 1, win], g_nc, start=False, stop=True
                            )
                        else:
                            q, lw = w // 4, w % 4
                            base = 2048 * q + 128 * lw
                            p_re = re_c[:, 512 * q + 128 * lw : 512 * q + 128 * lw + 128]
                            p_rs = rp[:, base + 1024 : base + 1152]
                            p_ic = rp[:, base + 1536 : base + 1664]
                            nc.tensor.matmul(
                                out, p_re, g_cd, start=True, stop=False
                            )
                            nc.tensor.matmul(
                                out, p_ic, g_nc, start=False, stop=False
                            )
                            nc.tensor.matmul(
                                out, p_rs, g_ncn, start=False, stop=True
                            )
                    evict_ab2(ps, sty3, g, "both" if g == 7 else ("act" if g % 2 == 0 else "dve"))

                # ---- phase B (strided windows) interleaved with phase C:
                # C chunk k depends only on B groups 2k,2k+1, so issue its
                # matmuls immediately after -- rotation products start ~3
                # groups earlier, hiding the rot chain under B's tail.
                def b_group(g):
                    ps = ps_ab.tile([P, 512], FP32, tag="ps_ab")
                    for j in range(2):
                        v = 2 * g + j
                        out = ps[:, 256 * j : 256 * (j + 1)]
                        nc.tensor.matmul(
                            out, sty_b[0][:, v], g_cd, start=True, stop=False
                        )
                        nc.tensor.matmul(
                            out, sty_b[1][:, v], g_nc, start=False, stop=True
                        )
                    evict_ab2(
                        ps, stx3, g,
                        "both" if g == 7 else ("dve" if g % 2 == 0 else "act"),
                    )

                def c_chunk(k):
                    pc = ps_c.tile([P, 1024], FP32, tag="ps_c")
                    ck = slice(512 * k, 512 * (k + 1))
                    re_m = stx3[:, 0, ck]
                    im_m = stx3[:, 1, ck]
                    nc.tensor.matmul(pc[:, 0:512], g_c4, re_m, start=True, stop=False)
                    nc.tensor.matmul(pc[:, 0:512], g_nd4, im_m, start=False, stop=True)
                    nc.tensor.matmul(
                        pc[:, 512:1024], g_d4, re_m, start=True, stop=False
                    )
                    nc.tensor.matmul(
                        pc[:, 512:1024], g_c4, im_m, start=False, stop=True
                    )

                    if t < 3:
                        cs = cs_tau if t % 2 == 0 else cs_id
                        cs_ap = cs[:, 1536 * k : 1536 * k + 1024]
                        sc_ap = cs[:, 1536 * k + 512 : 1536 * k + 1536]
                        sc = sc_pool.tile([P, 1024], FP16, tag="sc")
                        nc.scalar.copy(sc[:], pc[:])
                        rpk = slice(2048 * k, 2048 * k + 1024)
                        rpk2 = slice(2048 * k + 1024, 2048 * (k + 1))
                        if k == 3 and t > 0:
                            nc.gpsimd.tensor_tensor(
                                rp[:, rpk], sc[:], cs_ap, ALU.mult
                            )
                            nc.gpsimd.tensor_tensor(
                                rp[:, rpk2], sc[:], sc_ap, ALU.mult
                            )
                            nc.gpsimd.tensor_tensor(
                                re_c[:, 512 * k : 512 * (k + 1)],
                                rp[:, 2048 * k : 2048 * k + 512],
                                rp[:, 2048 * k + 512 : 2048 * k + 1024],
                                ALU.add,
                            )
                        else:
                            nc.vector.tensor_tensor(
                                rp[:, rpk], sc[:], cs_ap, ALU.mult
                            )
                            nc.vector.tensor_tensor(
                                rp[:, rpk2], sc[:], sc_ap, ALU.mult
                            )
                            nc.vector.tensor_tensor(
                                re_c[:, 512 * k : 512 * (k + 1)],
                                rp[:, 2048 * k : 2048 * k + 512],
                                rp[:, 2048 * k + 512 : 2048 * k + 1024],
                                ALU.add,
                            )
                    else:
                        sq = sc_pool.tile([P, 1024], FP16, tag="sc")
                        nc.scalar.activation(
                            sq[:], pc[:], mybir.ActivationFunctionType.Square
                        )
                        pr = pr_pool.tile([P, 2048], FP16, tag="pr")
                        nc.vector.tensor_tensor(
                            pr[:, 0:512], sq[:, 0:512], sq[:, 512:1024], ALU.add
                        )
                        nc.vector.scalar_tensor_tensor(
                            pr[:, 512:1024],
                            pr[:, 0:512],
                            1.0,
                            hp_d[:, ck],
                            ALU.mult,
                            ALU.mult,
                            accum_out=part_k[k][:],
                        )

                for g in range(8):
                    b_group(g)
                for k in range(4):
                    c_chunk(k)

            nc.vector.tensor_tensor(red0[:], part_k[0][:], part_k[1][:], ALU.add)
            nc.vector.tensor_tensor(red1[:], part_k[2][:], part_k[3][:], ALU.add)
            nc.vector.tensor_tensor(partial[:], red0[:], red1[:], ALU.add)
            nc.gpsimd.dma_start(d_out.ap(), partial[:])

    nc.compile()
    return nc


_NC_CACHE = {}


def _get_program():
    if "nc" not in _NC_CACHE:
        _NC_CACHE["nc"] = _build_program()
    return _NC_CACHE["nc"]


def kernel(batch_betas, adj_matrices, _trace=False, _tmpdir=None):
    batch_betas = np.asarray(batch_betas, dtype=np.float32)
    adj_matrices = np.asarray(adj_matrices, dtype=np.float32)
    assert batch_betas.shape == (BATCH, LAYERS)
    assert adj_matrices.shape == (BATCH, N, N)

    nc = _get_program()
    in_maps = _host_prep(batch_betas, adj_matrices)
    res = run_bass_kernel_spmd(
        nc,
        in_maps,
        list(range(NCORES)),
        trace=_trace,
        tmpdir=_tmpdir,
    )
    energies = np.array(
        [res.results[b]["out"].sum() * HP_SCALE / DIM for b in range(BATCH)],
        dtype=np.float32,
    )
    if _trace:
        return energies, res
    return energies


# revision 38
# speedup vs baseline: 1.1282x; 1.1282x over previous
"""TRN2 Bass kernel for nn_DiffQuantumSimulator (QAOA MaxCut, 18 qubits, p=4).

Data-parallel over batch (8 graphs -> 8 NeuronCores). Per core the 2^18
statevector lives in SBUF as fp16 [128 partitions x (2 planes x 2048)]
(re/im planes).

Each QAOA layer: diagonal phase exp(-i*hp) (elementwise) + mixer
RX(beta)^(x)18 done in 3 fp16 matmul stages (7+7+4 qubits).

Stages A and B use the state as the stationary operand, fusing a
partition<->free transpose into the matmul; complex arithmetic is two
accumulating matmuls per window against [C|D] and [-D|C] gate blocks.
Stage C applies kron(RX^4, I8) with the gate stationary. All PSUM
evictions write CONTIGUOUS SBUF ranges (strided SBUF writes are ~4x
slower); the layout permutation is carried by stage B's strided
stationary reads (8-element runs) instead. The per-layer bit
permutation tau (blocks 7-10 <-> 14-17) is an involution, so layouts
alternate tau/identity and the energy layout is the identity.

The diagonal rotation exp(-i*hp) is applied WITHOUT combine passes: one
PSUM->fp16 eviction + two fp16 product passes produce the four planes
(re*c, im*s, re*s, im*c), and the next layer's stage A consumes them
directly as four accumulating matmuls per window (matmul linearity; the
subtraction rides in a sign-flipped gate block [D|-C]). This removes the
rotation's exposed latency from the critical path entirely. Energy =
sum(|amp|^2 * hp/64) via ACT Square + DVE STT accumulate; host scales
by 64/DIM. Input DMAs are ordered by first use across the three serial
engine queues (~110GB/s each) so the first matmul starts ~1us after the
fixed ~9.5us framework startup.
"""

import numpy as np

import concourse.bass as bass
import concourse.mybir as mybir
import concourse.tile as tile
from concourse import bacc
from concourse.bass_utils import run_bass_kernel_spmd

N = 18
DIM = 1 << N
P = 128
F = 2048
LAYERS = 4
BATCH = 8
NCORES = 8
HP_SCALE = 64.0
GW = 1152  # per layer: [C7|D7] [-D7|C7] [D7|-C7] (3x256) C4 -D4 D4 (3x128)

FP32 = mybir.dt.float32
FP16 = mybir.dt.float16
ALU = mybir.AluOpType

# ----------------------------------------------------------------------------
# Host-side math
# ----------------------------------------------------------------------------


def _compute_hp(adj):
    W = (np.triu(adj, k=1) > 0.5).astype(np.float64)
    n_edges = W.sum()
    idx = np.arange(DIM)
    shifts = (N - 1 - np.arange(N))[:, None]
    Z = 1.0 - 2.0 * ((idx[None, :] >> shifts) & 1).astype(np.float64)
    T = W @ Z
    cross = np.einsum("ud,ud->d", T, Z)
    return 0.5 * (n_edges - cross)  # [DIM], integer-valued, exact


def _tau(b):
    """Per-layer bit-position permutation: A moves 11-17 -> 0-6 and 0-6 ->
    11-17 keeping 7-10; B (stride-16 windows, out part = col/16) then maps
    0-3 -> 7-10, 4-6 -> 11-13, 7-10 -> 14-17, 11-17 -> 0-6. Composition:"""
    if b <= 6:
        return b
    if b <= 10:
        return b + 7
    return b - 4


def _perm_for_power(s):
    pos = list(range(N))
    for _ in range(s):
        pos = [_tau(p) for p in pos]
    y = np.arange(DIM, dtype=np.int64)
    x = np.zeros(DIM, dtype=np.int64)
    for b in range(N):
        x |= ((y >> pos[b]) & 1) << b
    return x


_PERMS = [_perm_for_power(s) for s in range(5)]


def _rx(beta):
    c, s = np.cos(beta), np.sin(beta)
    return np.array([[c, -1j * s], [-1j * s, c]], dtype=np.complex128)


def _kron_list(mats):
    out = np.array([[1.0]], dtype=np.complex128)
    for m in mats:
        out = np.kron(out, m)
    return out


def _host_prep(batch_betas, adj_matrices):
    in_maps = []
    for b in range(BATCH):
        hp = _compute_hp(np.asarray(adj_matrices[b], dtype=np.float64))

        # init state: e^{-i hp} in identity layout (layer-1 rotation folded in)
        init = np.empty((P, 2, F), dtype=np.float64)
        init[:, 0, :] = np.cos(hp).reshape(P, F)
        init[:, 1, :] = (-np.sin(hp)).reshape(P, F)
        init = init.reshape(P, 2 * F).astype(np.float16)

        # rotation diags: layer 0 and 2 rotations share the tau layout,
        # layer 1 uses identity. Per chunk k: [cos|sin|cos] (1536 cols) so
        # (cos|sin) and (sin|cos) are both contiguous slices.
        def _csc(h):
            hl = h.reshape(P, 4, 512)
            c, s = np.cos(hl), np.sin(hl)
            return np.concatenate([c, s, c], axis=2).reshape(P, 6144).astype(np.float16)

        cs_tau = _csc(hp[_PERMS[1]])
        cs_id = _csc(hp)

        hp_fin = (hp[_PERMS[4]].reshape(P, F) / HP_SCALE).astype(np.float16)

        gates = np.empty((P, LAYERS * GW), dtype=np.float64)
        for t in range(LAYERS):
            beta = float(np.asarray(batch_betas[b][t], dtype=np.float64))
            M7 = _kron_list([_rx(beta)] * 7)
            C7, D7 = M7.real, M7.imag
            MC = np.kron(_kron_list([_rx(beta)] * 4), np.eye(8))
            C4, D4 = MC.real, MC.imag
            g = gates[:, GW * t : GW * (t + 1)]
            g[:, 0:128] = C7
            g[:, 128:256] = D7
            g[:, 256:384] = -D7
            g[:, 384:512] = C7
            g[:, 512:640] = D7
            g[:, 640:768] = -C7
            g[:, 768:896] = C4
            g[:, 896:1024] = -D4
            g[:, 1024:1152] = D4
        gates = gates.astype(np.float16)

        in_maps.append(
            {"init": init, "gates": gates, "cs_tau": cs_tau, "cs_id": cs_id,
             "hp_fin": hp_fin}
        )
    return in_maps


# ----------------------------------------------------------------------------
# Bass program
# ----------------------------------------------------------------------------


def _build_program():
    nc = bacc.Bacc("TRN2", target_bir_lowering=False, debug=False)

    d_init = nc.dram_tensor("init", [P, 2 * F], FP16, kind="ExternalInput")
    d_gates = nc.dram_tensor("gates", [P, LAYERS * GW], FP16, kind="ExternalInput")
    d_cs_tau = nc.dram_tensor("cs_tau", [P, 6144], FP16, kind="ExternalInput")
    d_cs_id = nc.dram_tensor("cs_id", [P, 6144], FP16, kind="ExternalInput")
    d_hp = nc.dram_tensor("hp_fin", [P, F], FP16, kind="ExternalInput")
    d_out = nc.dram_tensor("out", [P, 1], FP32, kind="ExternalOutput")

    with tile.TileContext(nc) as tc:
        with (
            tc.tile_pool(name="state", bufs=1) as st_pool,
            tc.tile_pool(name="consts", bufs=1) as c_pool,
            tc.tile_pool(name="sc", bufs=3) as sc_pool,
            tc.tile_pool(name="pr", bufs=3) as pr_pool,
            tc.tile_pool(name="misc", bufs=1) as m_pool,
            tc.tile_pool(name="ps_ab", bufs=4, space="PSUM") as ps_ab,
            tc.tile_pool(name="ps_c", bufs=2, space="PSUM") as ps_c,
        ):
            st_x = st_pool.tile([P, 2 * F], FP16, tag="st_x")
            st_y = st_pool.tile([P, 2 * F], FP16, tag="st_y")
            gates = c_pool.tile([P, LAYERS * GW], FP16, tag="gates")
            cs_tau = c_pool.tile([P, 6144], FP16, tag="cs_tau")
            cs_id = c_pool.tile([P, 6144], FP16, tag="cs_id")
            hp_d = c_pool.tile([P, F], FP16, tag="hp")
            rp = c_pool.tile([P, 8192], FP16, tag="rp")

            part_k = [
                m_pool.tile([P, 1], FP32, tag=f"part{k}", name=f"part{k}")
                for k in range(4)
            ]
            red0 = m_pool.tile([P, 1], FP32, tag="red0")
            red1 = m_pool.tile([P, 1], FP32, tag="red1")
            partial = m_pool.tile([P, 1], FP32, tag="partial")

            # ---- input DMAs: per-engine queues are serial (~110GB/s),
            # so order pieces by first use. The three critical first pieces
            # (init re k0, init im k0, layer-1 A-gates) go first on the three
            # queues; cs_tau is split three ways right behind.
            nc.sync.dma_start(st_x[:, 0:256], d_init.ap()[:, 0:256])
            nc.sync.dma_start(st_x[:, 256:1024], d_init.ap()[:, 256:1024])
            nc.sync.dma_start(st_x[:, 1024:2048], d_init.ap()[:, 1024:2048])
            nc.sync.dma_start(cs_tau[:, 0:2048], d_cs_tau.ap()[:, 0:2048])
            nc.sync.dma_start(cs_id[:, 0:3072], d_cs_id.ap()[:, 0:3072])
            nc.sync.dma_start(gates[:, GW : 2 * GW], d_gates.ap()[:, GW : 2 * GW])

            nc.scalar.dma_start(gates[:, 0:512], d_gates.ap()[:, 0:512])
            nc.scalar.dma_start(gates[:, 512:GW], d_gates.ap()[:, 512:GW])
            nc.scalar.dma_start(cs_tau[:, 2048:4096], d_cs_tau.ap()[:, 2048:4096])
            nc.scalar.dma_start(cs_id[:, 3072:6144], d_cs_id.ap()[:, 3072:6144])
            nc.scalar.dma_start(
                gates[:, 2 * GW : 4 * GW], d_gates.ap()[:, 2 * GW : 4 * GW]
            )
            nc.scalar.dma_start(hp_d[:], d_hp.ap())

            nc.gpsimd.dma_start(st_x[:, 2048:2304], d_init.ap()[:, 2048:2304])
            nc.gpsimd.dma_start(st_x[:, 2304:3072], d_init.ap()[:, 2304:3072])
            nc.gpsimd.dma_start(st_x[:, 3072:4096], d_init.ap()[:, 3072:4096])
            nc.gpsimd.dma_start(cs_tau[:, 4096:6144], d_cs_tau.ap()[:, 4096:6144])

            stx3 = st_x[:].rearrange("p (two c) -> p two c", two=2)
            sty3 = st_y[:].rearrange("p (two c) -> p two c", two=2)
            # phase-B stationary views: [p, v(16; stride 1), j(128; stride 16)]
            sty_b = [
                st_y[:, 2048 * pl : 2048 * (pl + 1)].rearrange(
                    "p (j v) -> p v j", v=16
                )
                for pl in (0, 1)
            ]

            def evict_ab2(src_ps, dst3, g, engine):
                """[P,512] PSUM group (2 windows) -> contiguous cols
                256g+128j+n. engine='both' splits re/im across ACT+DVE to
                minimize the barrier tail on the last group."""
                src = src_ps[:].rearrange("p (j two n) -> p two j n", j=2, two=2)
                dst = dst3[:, :, 256 * g : 256 * (g + 1)].rearrange(
                    "p two (j n) -> p two j n", j=2
                )
                if engine == "act":
                    nc.scalar.copy(dst, src)
                elif engine == "dve":
                    nc.vector.tensor_copy(dst, src)
                else:
                    nc.scalar.copy(dst[:, 0], src[:, 0])
                    nc.vector.tensor_copy(dst[:, 1], src[:, 1])

            # Pool warm-up: force the compute library load early
            nc.gpsimd.tensor_tensor(red0[:], part_k[0][:], part_k[0][:], ALU.add)

            for t in range(LAYERS):
                gb = GW * t
                g_cd = gates[:, gb : gb + 256]
                g_nc = gates[:, gb + 256 : gb + 512]
                g_ncn = gates[:, gb + 512 : gb + 768]
                g_c4 = gates[:, gb + 768 : gb + 896]
                g_nd4 = gates[:, gb + 896 : gb + 1024]
                g_d4 = gates[:, gb + 1024 : gb + 1152]

                # ---- phase A: stationary = state windows (fused transpose).
                # Layer 0 reads the init state (2 matmuls/window); later
                # layers read the uncombined rotation product planes
                # (4 accumulating matmuls/window) so no combine pass exists.
                for g in range(8):
                    ps = ps_ab.tile([P, 512], FP32, tag="ps_ab")
                    for j in range(2):
                        w = 2 * g + j
                        out = ps[:, 256 * j : 256 * (j + 1)]
                        if t == 0:
                            win = slice(128 * w, 128 * (w + 1))
                            nc.tensor.matmul(
                                out, stx3[:, 0, win], g_cd, start=True, stop=False
                            )
                            nc.tensor.matmul(
                                out, stx3[:, 1, win], g_nc, start=False, stop=True
                            )
                        else:
                            q, lw = w // 4, w % 4
                            base = 2048 * q + 128 * lw
                            p_rc = rp[:, base : base + 128]
                            p_is = rp[:, base + 512 : base + 640]
                            p_rs = rp[:, base + 1024 : base + 1152]
                            p_ic = rp[:, base + 1536 : base + 1664]
                            nc.tensor.matmul(
                                out, p_rc, g_cd, start=True, stop=False
                            )
                            nc.tensor.matmul(
                                out, p_is, g_cd, start=False, stop=False
                            )
                            nc.tensor.matmul(
                                out, p_ic, g_nc, start=False, stop=False
                            )
                            nc.tensor.matmul(
                                out, p_rs, g_ncn, start=False, stop=True
                            )
                    evict_ab2(ps, sty3, g, "both" if g == 7 else ("act" if g % 2 == 0 else "dve"))

                # ---- phase B (strided windows) interleaved with phase C:
                # C chunk k depends only on B groups 2k,2k+1, so issue its
                # matmuls immediately after -- rotation products start ~3
                # groups earlier, hiding the rot chain under B's tail.
                def b_group(g):
                    ps = ps_ab.tile([P, 512], FP32, tag="ps_ab")
                    for j in range(2):
                        v = 2 * g + j
                        out = ps[:, 256 * j : 256 * (j + 1)]
                        nc.tensor.matmul(
                            out, sty_b[0][:, v], g_cd, start=True, stop=False
                        )
                        nc.tensor.matmul(
                            out, sty_b[1][:, v], g_nc, start=False, stop=True
                        )
                    evict_ab2(
                        ps, stx3, g,
                        "both" if g == 7 else ("dve" if g % 2 == 0 else "act"),
                    )

                def c_chunk(k):
                    pc = ps_c.tile([P, 1024], FP32, tag="ps_c")
                    ck = slice(512 * k, 512 * (k + 1))
                    re_m = stx3[:, 0, ck]
                    im_m = stx3[:, 1, ck]
                    nc.tensor.matmul(pc[:, 0:512], g_c4, re_m, start=True, stop=False)
                    nc.tensor.matmul(pc[:, 0:512], g_nd4, im_m, start=False, stop=True)
                    nc.tensor.matmul(
                        pc[:, 512:1024], g_d4, re_m, start=True, stop=False
                    )
                    nc.tensor.matmul(
                        pc[:, 512:1024], g_c4, im_m, start=False, stop=True
                    )

                    if t < 3:
                        cs = cs_tau if t % 2 == 0 else cs_id
                        cs_ap = cs[:, 1536 * k : 1536 * k + 1024]
                        sc_ap = cs[:, 1536 * k + 512 : 1536 * k + 1536]
                        sc = sc_pool.tile([P, 1024], FP16, tag="sc")
                        nc.scalar.copy(sc[:], pc[:])
                        rpk = slice(2048 * k, 2048 * k + 1024)
                        rpk2 = slice(2048 * k + 1024, 2048 * (k + 1))
                        if k == 3 and t > 0:
                            nc.gpsimd.tensor_tensor(
                                rp[:, rpk], sc[:], cs_ap, ALU.mult
                            )
                            nc.gpsimd.tensor_tensor(
                                rp[:, rpk2], sc[:], sc_ap, ALU.mult
                            )
                        else:
                            nc.vector.tensor_tensor(
                                rp[:, rpk], sc[:], cs_ap, ALU.mult
                            )
                            nc.vector.tensor_tensor(
                                rp[:, rpk2], sc[:], sc_ap, ALU.mult
                            )
                    else:
                        sq = sc_pool.tile([P, 1024], FP16, tag="sc")
                        nc.scalar.activation(
                            sq[:], pc[:], mybir.ActivationFunctionType.Square
                        )
                        pr = pr_pool.tile([P, 2048], FP16, tag="pr")
                        nc.vector.tensor_tensor(
                            pr[:, 0:512], sq[:, 0:512], sq[:, 512:1024], ALU.add
                        )
                        nc.vector.scalar_tensor_tensor(
                            pr[:, 512:1024],
                            pr[:, 0:512],
                            1.0,
                            hp_d[:, ck],
                            ALU.mult,
                            ALU.mult,
                            accum_out=part_k[k][:],
                        )

                for g in range(8):
                    b_group(g)
                for k in range(4):
                    c_chunk(k)

            nc.vector.tensor_tensor(red0[:], part_k[0][:], part_k[1][:], ALU.add)
            nc.vector.tensor_tensor(red1[:], part_k[2][:], part_k[3][:], ALU.add)
            nc.vector.tensor_tensor(partial[:], red0[:], red1[:], ALU.add)
            nc.gpsimd.dma_start(d_out.ap(), partial[:])

    nc.compile()
    return nc


_NC_CACHE = {}


def _get_program():
    if "nc" not in _NC_CACHE:
        _NC_CACHE["nc"] = _build_program()
    return _NC_CACHE["nc"]


def kernel(batch_betas, adj_matrices, _trace=False, _tmpdir=None):
    batch_betas = np.asarray(batch_betas, dtype=np.float32)
    adj_matrices = np.asarray(adj_matrices, dtype=np.float32)
    assert batch_betas.shape == (BATCH, LAYERS)
    assert adj_matrices.shape == (BATCH, N, N)

    nc = _get_program()
    in_maps = _host_prep(batch_betas, adj_matrices)
    res = run_bass_kernel_spmd(
        nc,
        in_maps,
        list(range(NCORES)),
        trace=_trace,
        tmpdir=_tmpdir,
    )
    energies = np.array(
        [res.results[b]["out"].sum() * HP_SCALE / DIM for b in range(BATCH)],
        dtype=np.float32,
    )
    if _trace:
        return energies, res
    return energies
